# revision 19
# baseline (speedup 1.0000x reference)
"""Trainium2 Bass kernel for the real-space Ewald sum (nn_Ewald).

Math (per molecule b, nb=2048 atoms, 8 charge channels):
    pot_b = sum_{i,j} qq_ij * erf(|rij|/sqrt(2)) / (|rij|+1e-6) / (4*pi)
          + sum_i qq_ii / (2*pi)^1.5            (self term)
    all scaled by NORM_FACTOR.

Kernel formulation per pair tile:
    s_ij  = |ri|^2 + |rj|^2 - 2 ri.rj        (K=13 bf16 hi/lo augmented matmul;
                                              bf16 x bf16 products are exact in
                                              the fp32 PSUM accumulator)
    y     = abs_rsqrt(s)                      (ACT table)
    d     = max(|s| * min(y, 1e4), 1e-4)      (custom DVE op; == sqrt(|s|))
    e     = erf(d / sqrt(2))                  (ACT table)
    w     = (idx != diag) * e * min(y, 1e4)   (custom DVE op; exact 0 diagonal)
    u[c,j] += lam * sum_i q[i,c] w_ij         (PE f32r contraction, PSUM accum;
                                              lam in {1,2} via q vs 2q stationary)
Host: pot_b = sum(u * q^T)/(4*pi) + self term, * NORM.

Symmetry: w is symmetric, so only the block upper triangle is computed.
Row block rb (0..15 within molecule, 128 atoms) covers col windows
jc >= rb//4; the diagonal 512-block gets weight 1 (both orientations of
in-group pairs are computed), strictly-above blocks get weight 2. Each
core takes 8 row blocks whose triangle widths form the multiset
{4,4,3,3,2,2,1,1} so all 8 cores run the identical program (SPMD).

Sharding: 8 cores = 4 molecules x 2 row-block sets.
"""

import numpy as np

B = 4
NB = 2048
NQ = 8
NCORES = 8
RBM = 16            # row blocks per molecule
RB = 8              # row blocks per core
CT = 512            # matmul col tile (PSUM bank)
CAP = 1e4
FLOOR = 1e-4
INV_SQRT2 = 0.7071067811865476
NORM_FACTOR = 90.0474
KA = 13             # augmented contraction depth (bf16 hi/lo split)

# per-slot triangle width in 512-blocks; identical multiset on every core
NJC = [4, 4, 3, 3, 2, 2, 1, 1]
# row blocks (within molecule) per core half
SLOT_RBG = {
    0: [0, 1, 4, 5, 8, 9, 12, 13],
    1: [2, 3, 6, 7, 10, 11, 14, 15],
}
# chunk widths (ACT/DVE op granularity) per NJC: one PSUM bank each
CHUNKS = {n: [CT] * n for n in (1, 2, 3, 4)}
# erf groups: lists of slots per erf call
ERF_GROUPS = [[0], [1], [2, 3], [4, 5, 6, 7]]
# last slot contributing to each u bank (first is always slot 0)
BANK_LAST_SLOT = {0: 1, 1: 3, 2: 5, 3: 7}

# packed Y/D layout: per-slot window offsets
SLOT_W = [n * CT for n in NJC]
SLOT_OFF = np.concatenate([[0], np.cumsum(SLOT_W)]).tolist()
TOTW = SLOT_OFF[-1]  # 10240
N_CHUNKS = sum(len(CHUNKS[n]) for n in NJC)  # 12

_compiled = None
_ops = None


def _register_ops():
    """Register the two custom DVE ops (idempotent)."""
    global _ops
    if _ops is not None:
        return _ops
    from concourse import dve_ops
    from concourse.dve_spec import (
        Spec, Src0, Src1, C0, C1, Zero, maxx, minn, lower, _has_src1,
        Idx, ne,
    )
    from concourse.dve_uop import DveOpSpec

    def mk(name, spec):
        for o in dve_ops.OPS:
            if o.name == name:
                return o
        shas = {}
        for ver in ("v3", "v4"):
            tmp = DveOpSpec(
                name=name,
                opcode=31,
                uops=lower(spec, ver=ver),
                rd1_en=_has_src1(spec),
            )
            shas[ver] = tmp.sha(ver)
        op = dve_ops.DveOp(name, spec, subdim=False, uops_sha=shas)
        dve_ops.OPS.append(op)
        dve_ops._SUB_OPCODE_FOR_NAME[name] = (
            dve_ops._CUSTOM_DVE_ROW_BASE + len(dve_ops.OPS) - 1
        )
        dve_ops.CUSTOM_DVE_SPECS[name] = spec
        return op

    # d = max(|s| * min(y, cap), floor)
    ewald_d = mk(
        "EWALD_D",
        Spec(
            body=maxx(maxx(Src0, Zero - Src0) * minn(Src1, C0), C1),
            reference=lambda in0, in1, s0, s1, imm2: np.maximum(
                np.abs(in0.astype(np.float32))
                * np.minimum(in1.astype(np.float32), np.float32(s0)),
                np.float32(s1),
            ).astype(np.float32),
        ),
    )

    # w = (idx != diag) * e * min(y, cap)
    def _w_ref(in0, in1, s0, s1, imm2):
        in0 = in0.astype(np.float32).reshape(in0.shape[0], -1)
        in1 = in1.astype(np.float32).reshape(in0.shape)
        idx = np.broadcast_to(
            np.arange(in0.shape[1], dtype=np.float32), in0.shape
        )
        dval = np.asarray(s0, np.float32).reshape(-1, 1)
        w = in0 * np.minimum(in1, np.float32(s1))
        return np.where(idx == dval, np.float32(0.0), w).astype(np.float32)

    ewald_w = mk(
        "EWALD_W2",
        Spec(
            body=ne(Idx, C0) * Src0 * minn(Src1, C1),
            reference=_w_ref,
        ),
    )
    _ops = (ewald_d, ewald_w)
    return _ops


def build_nc(psa_bufs=4):
    """Build + compile the per-core Bass program (fixed shapes)."""
    from concourse import bacc, tile
    import concourse.mybir as mybir
    from concourse.bass import ts, ds
    from concourse.tile_rust import add_dep_helper

    ewald_d, ewald_w = _register_ops()
    f32 = mybir.dt.float32
    f32r = mybir.dt.float32r
    bf16 = mybir.dt.bfloat16
    AF = mybir.ActivationFunctionType

    nc = bacc.Bacc(
        "TRN2",
        target_bir_lowering=False,
        debug=False,
        num_devices=NCORES,
    )
    # augL columns are slot-ordered (host maps slots -> molecule row blocks)
    augL = nc.dram_tensor("augL", [KA, RB * 128], bf16, kind="ExternalInput").ap()
    augR = nc.dram_tensor("augR", [KA, NB], bf16, kind="ExternalInput").ap()
    q1 = nc.dram_tensor("q1", [128, RB * NQ], f32r, kind="ExternalInput").ap()
    q2 = nc.dram_tensor("q2", [128, RB * NQ], f32r, kind="ExternalInput").ap()
    diag = nc.dram_tensor("diag", [128, N_CHUNKS], f32, kind="ExternalInput").ap()
    uout = nc.dram_tensor("uout", [NQ, NB], f32, kind="ExternalOutput").ap()

    with tile.TileContext(nc) as tc:
        with (
            tc.tile_pool(name="const", bufs=1) as cpool,
            tc.tile_pool(name="work", bufs=6) as wpool,
            tc.tile_pool(name="big", bufs=1) as bigpool,
            tc.tile_pool(name="psA", bufs=psa_bufs, space="PSUM") as psA,
            tc.tile_pool(name="psU", bufs=1, space="PSUM") as psU,
        ):
            augR_sb = cpool.tile([KA, NB], bf16)
            nc.sync.dma_start(out=augR_sb[:], in_=augR[:])
            augL_sb = cpool.tile([KA, RB * 128], bf16)
            nc.sync.dma_start(out=augL_sb[:], in_=augL[:])
            q1_sb = cpool.tile([128, RB * NQ], f32r)
            nc.gpsimd.dma_start(out=q1_sb[:], in_=q1[:])
            q2_sb = cpool.tile([128, RB * NQ], f32r)
            nc.gpsimd.dma_start(out=q2_sb[:], in_=q2[:])
            diag_sb = cpool.tile([128, N_CHUNKS], f32)
            nc.gpsimd.dma_start(out=diag_sb[:], in_=diag[:])

            Y = bigpool.tile([128, TOTW], f32)
            D = bigpool.tile([128, TOTW], f32)

            # ---- Phase A: s -> y, d  (ACT table: abs_reciprocal_sqrt) ----
            last_rsqrt = None
            chunk_idx = 0
            chunk_of = {}  # (slot, ci) -> global chunk index
            for t in range(RB):
                col0 = NB - NJC[t] * CT  # window start col in molecule
                coff = 0
                for ci, cw in enumerate(CHUNKS[NJC[t]]):
                    chunk_of[(t, ci)] = chunk_idx
                    chunk_idx += 1
                    off = SLOT_OFF[t] + coff
                    s_ps = psA.tile([128, cw], f32, tag="s_ps")
                    for hminor in range(cw // CT):
                        nc.tensor.matmul(
                            s_ps[:, ts(hminor, CT)],
                            augL_sb[:, ts(t, 128)],
                            augR_sb[:, ds(col0 + coff + hminor * CT, CT)],
                            start=True,
                            stop=True,
                        )
                    last_rsqrt = nc.scalar.activation(
                        Y[:, ds(off, cw)], s_ps[:], AF.Abs_reciprocal_sqrt
                    )
                    nc.vector._custom_dve(
                        ewald_d,
                        out=D[:, ds(off, cw)],
                        in0=s_ps[:],
                        in1=Y[:, ds(off, cw)],
                        s0=CAP,
                        s1=FLOOR,
                    )
                    coff += cw

            # ---- Phase B: erf groups (order-pinned after all rsqrts) ----
            u_ps = psU.tile([NQ, 4 * CT], f32)
            for group in ERF_GROUPS:
                goff = SLOT_OFF[group[0]]
                gw = sum(SLOT_W[t] for t in group)
                einst = nc.scalar.activation(
                    D[:, ds(goff, gw)],
                    D[:, ds(goff, gw)],
                    AF.Erf,
                    scale=INV_SQRT2,
                )
                add_dep_helper(
                    einst.ins, last_rsqrt.ins, sync=False,
                    reason="ACT table set order: all rsqrt before any erf",
                )
                for t in group:
                    jc0 = 4 - NJC[t]  # first (diagonal) 512-block of window
                    coff = 0
                    for ci, cw in enumerate(CHUNKS[NJC[t]]):
                        off = SLOT_OFF[t] + coff
                        w = wpool.tile([128, cw], f32r, tag="w")
                        nc.vector._custom_dve(
                            ewald_w,
                            out=w[:],
                            in0=D[:, ds(off, cw)],
                            in1=Y[:, ds(off, cw)],
                            s0=diag_sb[:, ds(chunk_of[(t, ci)], 1)],
                            s1=CAP,
                        )
                        for hminor in range(cw // CT):
                            jc = jc0 + coff // CT + hminor
                            qs = q1_sb if jc == jc0 else q2_sb
                            nc.tensor.matmul(
                                u_ps[:, ts(jc, CT)],
                                qs[:, ds(t * NQ, NQ)],
                                w[:, ts(hminor, CT)],
                                start=(t == 0),
                                stop=(t == BANK_LAST_SLOT[jc]),
                            )
                        coff += cw

            for jc in range(4):
                u_sb = wpool.tile([NQ, CT], f32, tag="u_sb")
                if jc % 2 == 0:
                    nc.scalar.copy(u_sb[:], u_ps[:, ts(jc, CT)])
                else:
                    nc.vector.tensor_copy(u_sb[:], u_ps[:, ts(jc, CT)])
                nc.sync.dma_start(out=uout[:, ts(jc, CT)], in_=u_sb[:])

    nc.compile()
    return nc


def make_in_maps(q, r):
    """Host-side sharding: per-core augmented bf16 hi/lo matrices."""
    import ml_dtypes

    bf = ml_dtypes.bfloat16
    q = np.ascontiguousarray(np.asarray(q, np.float32))
    r = np.ascontiguousarray(np.asarray(r, np.float32))
    in_maps = []
    for core in range(NCORES):
        b, h = core // 2, core % 2
        rm = r[b * NB : (b + 1) * NB]
        qm = q[b * NB : (b + 1) * NB]
        rc = (rm - rm.mean(0, keepdims=True)).astype(np.float32)
        hi = rc.astype(bf)
        lo = (rc - hi.astype(np.float32)).astype(bf)
        rr = hi.astype(np.float32) + lo.astype(np.float32)
        n2 = (rr * rr).sum(1).astype(np.float32)
        n2_hi = n2.astype(bf)
        n2_lo = (n2 - n2_hi.astype(np.float32)).astype(bf)

        rbgs = SLOT_RBG[h]
        rowsel = np.concatenate(
            [np.arange(g * 128, (g + 1) * 128) for g in rbgs]
        )
        ones_i = np.ones(RB * 128, bf)
        ones_j = np.ones(NB, bf)
        rowsL, rowsR = [], []
        for ax in range(3):
            rowsL += [hi[rowsel, ax], hi[rowsel, ax], lo[rowsel, ax]]
            rowsR += [
                (-2.0 * hi[:, ax].astype(np.float32)).astype(bf),
                (-2.0 * lo[:, ax].astype(np.float32)).astype(bf),
                (-2.0 * hi[:, ax].astype(np.float32)).astype(bf),
            ]
        rowsL += [n2_hi[rowsel], n2_lo[rowsel], ones_i, ones_i]
        rowsR += [ones_j, ones_j, n2_hi, n2_lo]
        augL_np = np.ascontiguousarray(np.stack(rowsL).astype(bf))
        augR_np = np.ascontiguousarray(np.stack(rowsR).astype(bf))

        qi = qm[rowsel]  # [RB*128, NQ] slot-ordered
        q1_np = np.ascontiguousarray(
            qi.reshape(RB, 128, NQ).transpose(1, 0, 2).reshape(128, RB * NQ)
        ).astype(np.float32)
        q2_np = np.ascontiguousarray(2.0 * q1_np)

        # diag index per chunk: the diagonal sits in chunk 0 of each slot
        # at within-window index 128*(rbg mod 4) + p.
        diag_np = np.full((128, N_CHUNKS), -1.0, np.float32)
        p = np.arange(128, dtype=np.float32)
        cidx = 0
        for t in range(RB):
            g = rbgs[t]
            for ci in range(len(CHUNKS[NJC[t]])):
                if ci == 0:
                    diag_np[:, cidx] = 128.0 * (g % 4) + p
                cidx += 1
        in_maps.append(
            {
                "augL": augL_np,
                "augR": augR_np,
                "q1": q1_np,
                "q2": q2_np,
                "diag": diag_np,
            }
        )
    return in_maps


def reduce_outputs(q, results):
    """Host-side gather: u[8,2048] per core -> pot[B].

    The kernel zeroes the diagonal exactly (select), so the self term
    sum(q^2)/(2*pi)^1.5 is added here in f64.
    """
    q = np.asarray(q, np.float32)
    pots = np.zeros(B, np.float64)
    for core in range(NCORES):
        b = core // 2
        u = results[core]["uout"].astype(np.float64)
        qm = q[b * NB : (b + 1) * NB].astype(np.float64)
        pots[b] += (u * qm.T).sum()
    pots = pots / (4.0 * np.pi)
    for b in range(B):
        qm = q[b * NB : (b + 1) * NB].astype(np.float64)
        pots[b] += (qm**2).sum() / ((2.0 * np.pi) ** 1.5)
    return (pots * NORM_FACTOR).astype(np.float32)


def kernel(q, r, batch):
    global _compiled
    if _compiled is None:
        try:
            _compiled = build_nc(psa_bufs=4)
        except Exception:
            _compiled = build_nc(psa_bufs=2)
    from concourse import bass_utils

    in_maps = make_in_maps(q, r)
    last_err = None
    for attempt in range(3):
        try:
            res = bass_utils.run_bass_kernel_spmd(
                _compiled, in_maps, core_ids=list(range(NCORES))
            )
            return reduce_outputs(q, res.results)
        except Exception as e:  # transient device errors: back off and retry
            last_err = e
            import time

            time.sleep(15 * (attempt + 1))
    raise last_err


# revision 20
# speedup vs baseline: 1.0126x; 1.0126x over previous
"""Trainium2 Bass kernel for the real-space Ewald sum (nn_Ewald).

Math (per molecule b, nb=2048 atoms, 8 charge channels):
    pot_b = sum_{i,j} qq_ij * erf(|rij|/sqrt(2)) / (|rij|+1e-6) / (4*pi)
          + sum_i qq_ii / (2*pi)^1.5            (self term)
    all scaled by NORM_FACTOR.

Kernel formulation per pair tile:
    s_ij  = |ri|^2 + |rj|^2 - 2 ri.rj        (K=13 bf16 hi/lo augmented matmul;
                                              bf16 x bf16 products are exact in
                                              the fp32 PSUM accumulator)
    y     = abs_rsqrt(s)                      (ACT table)
    d     = max(|s| * min(y, 1e4), 1e-4)      (custom DVE op; == sqrt(|s|))
    e     = erf(d / sqrt(2))                  (ACT table)
    w     = (idx != diag) * e * min(y, 1e4)   (custom DVE op; exact 0 diagonal)
    u[c,j] += lam * sum_i q[i,c] w_ij         (PE f32r contraction, PSUM accum;
                                              lam in {1,2} via q vs 2q stationary)
Host: pot_b = sum(u * q^T)/(4*pi) + self term, * NORM.

Symmetry: w is symmetric, so only the block upper triangle is computed.
Row block rb (0..15 within molecule, 128 atoms) covers col windows
jc >= rb//4; the diagonal 512-block gets weight 1 (both orientations of
in-group pairs are computed), strictly-above blocks get weight 2. Each
core takes 8 row blocks whose triangle widths form the multiset
{4,4,3,3,2,2,1,1} so all 8 cores run the identical program (SPMD).

Sharding: 8 cores = 4 molecules x 2 row-block sets.
"""

import numpy as np

B = 4
NB = 2048
NQ = 8
NCORES = 8
RBM = 16            # row blocks per molecule
RB = 8              # row blocks per core
CT = 512            # matmul col tile (PSUM bank)
CAP = 1e4
FLOOR = 1e-4
INV_SQRT2 = 0.7071067811865476
NORM_FACTOR = 90.0474
KA = 13             # augmented contraction depth (bf16 hi/lo split)

# per-slot triangle width in 512-blocks; identical multiset on every core
NJC = [4, 4, 3, 3, 2, 2, 1, 1]
# row blocks (within molecule) per core half
SLOT_RBG = {
    0: [0, 1, 4, 5, 8, 9, 12, 13],
    1: [2, 3, 6, 7, 10, 11, 14, 15],
}
# chunk widths (ACT/DVE op granularity) per NJC: one PSUM bank each
CHUNKS = {n: [CT] * n for n in (1, 2, 3, 4)}
# erf groups: lists of slots per erf call
ERF_GROUPS = [[0], [1], [2, 3], [4, 5, 6, 7]]
# last slot contributing to each u bank (first is always slot 0)
BANK_LAST_SLOT = {0: 1, 1: 3, 2: 5, 3: 7}

# packed Y/D layout: per-slot window offsets
SLOT_W = [n * CT for n in NJC]
SLOT_OFF = np.concatenate([[0], np.cumsum(SLOT_W)]).tolist()
TOTW = SLOT_OFF[-1]  # 10240
N_CHUNKS = sum(len(CHUNKS[n]) for n in NJC)  # 12

_compiled = None
_ops = None


def _register_ops():
    """Register the two custom DVE ops (idempotent)."""
    global _ops
    if _ops is not None:
        return _ops
    from concourse import dve_ops
    from concourse.dve_spec import (
        Spec, Src0, Src1, C0, C1, Zero, maxx, minn, lower, _has_src1,
        Idx, ne,
    )
    from concourse.dve_uop import DveOpSpec

    def mk(name, spec):
        for o in dve_ops.OPS:
            if o.name == name:
                return o
        shas = {}
        for ver in ("v3", "v4"):
            tmp = DveOpSpec(
                name=name,
                opcode=31,
                uops=lower(spec, ver=ver),
                rd1_en=_has_src1(spec),
            )
            shas[ver] = tmp.sha(ver)
        op = dve_ops.DveOp(name, spec, subdim=False, uops_sha=shas)
        dve_ops.OPS.append(op)
        dve_ops._SUB_OPCODE_FOR_NAME[name] = (
            dve_ops._CUSTOM_DVE_ROW_BASE + len(dve_ops.OPS) - 1
        )
        dve_ops.CUSTOM_DVE_SPECS[name] = spec
        return op

    # d = max(|s| * min(y, cap), floor)
    ewald_d = mk(
        "EWALD_D",
        Spec(
            body=maxx(maxx(Src0, Zero - Src0) * minn(Src1, C0), C1),
            reference=lambda in0, in1, s0, s1, imm2: np.maximum(
                np.abs(in0.astype(np.float32))
                * np.minimum(in1.astype(np.float32), np.float32(s0)),
                np.float32(s1),
            ).astype(np.float32),
        ),
    )

    # w = (idx != diag) * e * min(y, cap)
    def _w_ref(in0, in1, s0, s1, imm2):
        in0 = in0.astype(np.float32).reshape(in0.shape[0], -1)
        in1 = in1.astype(np.float32).reshape(in0.shape)
        idx = np.broadcast_to(
            np.arange(in0.shape[1], dtype=np.float32), in0.shape
        )
        dval = np.asarray(s0, np.float32).reshape(-1, 1)
        w = in0 * np.minimum(in1, np.float32(s1))
        return np.where(idx == dval, np.float32(0.0), w).astype(np.float32)

    ewald_w = mk(
        "EWALD_W2",
        Spec(
            body=ne(Idx, C0) * Src0 * minn(Src1, C1),
            reference=_w_ref,
        ),
    )
    _ops = (ewald_d, ewald_w)
    return _ops


def build_nc(psa_bufs=4):
    """Build + compile the per-core Bass program (fixed shapes)."""
    from concourse import bacc, tile
    import concourse.mybir as mybir
    from concourse.bass import ts, ds
    from concourse.tile_rust import add_dep_helper

    ewald_d, ewald_w = _register_ops()
    f32 = mybir.dt.float32
    f32r = mybir.dt.float32r
    bf16 = mybir.dt.bfloat16
    AF = mybir.ActivationFunctionType

    nc = bacc.Bacc(
        "TRN2",
        target_bir_lowering=False,
        debug=False,
        num_devices=NCORES,
    )
    # augL columns are slot-ordered (host maps slots -> molecule row blocks)
    augL = nc.dram_tensor("augL", [KA, RB * 128], bf16, kind="ExternalInput").ap()
    augR = nc.dram_tensor("augR", [KA, NB], bf16, kind="ExternalInput").ap()
    q1 = nc.dram_tensor("q1", [128, RB * NQ], f32r, kind="ExternalInput").ap()
    q2 = nc.dram_tensor("q2", [128, RB * NQ], f32r, kind="ExternalInput").ap()
    diag = nc.dram_tensor("diag", [128, N_CHUNKS], f32, kind="ExternalInput").ap()
    uout = nc.dram_tensor("uout", [NQ, NB], f32, kind="ExternalOutput").ap()

    with tile.TileContext(nc) as tc:
        with (
            tc.tile_pool(name="const", bufs=1) as cpool,
            tc.tile_pool(name="work", bufs=4) as wpool,
            tc.tile_pool(name="big", bufs=1) as bigpool,
            tc.tile_pool(name="psA", bufs=psa_bufs, space="PSUM") as psA,
            tc.tile_pool(name="psU", bufs=1, space="PSUM") as psU,
        ):
            augR_sb = cpool.tile([KA, NB], bf16)
            nc.sync.dma_start(out=augR_sb[:], in_=augR[:])
            augL_sb = cpool.tile([KA, RB * 128], bf16)
            nc.sync.dma_start(out=augL_sb[:], in_=augL[:])
            q1_sb = cpool.tile([128, RB * NQ], f32r)
            nc.gpsimd.dma_start(out=q1_sb[:], in_=q1[:])
            q2_sb = cpool.tile([128, RB * NQ], f32r)
            nc.gpsimd.dma_start(out=q2_sb[:], in_=q2[:])
            diag_sb = cpool.tile([128, N_CHUNKS], f32)
            nc.gpsimd.dma_start(out=diag_sb[:], in_=diag[:])

            Y = bigpool.tile([128, TOTW], f32)
            D = bigpool.tile([128, TOTW], f32)

            # ---- Phase A: s -> y, d  (ACT table: abs_reciprocal_sqrt) ----
            last_rsqrt = None
            chunk_idx = 0
            chunk_of = {}  # (slot, ci) -> global chunk index
            for t in range(RB):
                col0 = NB - NJC[t] * CT  # window start col in molecule
                coff = 0
                for ci, cw in enumerate(CHUNKS[NJC[t]]):
                    chunk_of[(t, ci)] = chunk_idx
                    chunk_idx += 1
                    off = SLOT_OFF[t] + coff
                    s_ps = psA.tile([128, cw], f32, tag="s_ps")
                    for hminor in range(cw // CT):
                        nc.tensor.matmul(
                            s_ps[:, ts(hminor, CT)],
                            augL_sb[:, ts(t, 128)],
                            augR_sb[:, ds(col0 + coff + hminor * CT, CT)],
                            start=True,
                            stop=True,
                        )
                    last_rsqrt = nc.scalar.activation(
                        Y[:, ds(off, cw)], s_ps[:], AF.Abs_reciprocal_sqrt
                    )
                    nc.vector._custom_dve(
                        ewald_d,
                        out=D[:, ds(off, cw)],
                        in0=s_ps[:],
                        in1=Y[:, ds(off, cw)],
                        s0=CAP,
                        s1=FLOOR,
                    )
                    coff += cw

            # ---- Phase B: erf groups (order-pinned after all rsqrts) ----
            u_ps = psU.tile([NQ, 4 * CT], f32)
            for group in ERF_GROUPS:
                goff = SLOT_OFF[group[0]]
                gw = sum(SLOT_W[t] for t in group)
                einst = nc.scalar.activation(
                    D[:, ds(goff, gw)],
                    D[:, ds(goff, gw)],
                    AF.Erf,
                    scale=INV_SQRT2,
                )
                add_dep_helper(
                    einst.ins, last_rsqrt.ins, sync=False,
                    reason="ACT table set order: all rsqrt before any erf",
                )
                for t in group:
                    jc0 = 4 - NJC[t]  # first (diagonal) 512-block of window
                    coff = 0
                    for ci, cw in enumerate(CHUNKS[NJC[t]]):
                        off = SLOT_OFF[t] + coff
                        w = wpool.tile([128, cw], f32r, tag="w")
                        nc.vector._custom_dve(
                            ewald_w,
                            out=w[:],
                            in0=D[:, ds(off, cw)],
                            in1=Y[:, ds(off, cw)],
                            s0=diag_sb[:, ds(chunk_of[(t, ci)], 1)],
                            s1=CAP,
                        )
                        for hminor in range(cw // CT):
                            jc = jc0 + coff // CT + hminor
                            qs = q1_sb if jc == jc0 else q2_sb
                            nc.tensor.matmul(
                                u_ps[:, ts(jc, CT)],
                                qs[:, ds(t * NQ, NQ)],
                                w[:, ts(hminor, CT)],
                                start=(t == 0),
                                stop=(t == BANK_LAST_SLOT[jc]),
                            )
                        coff += cw

            for jc in range(4):
                u_sb = wpool.tile([NQ, CT], f32, tag="u_sb")
                if jc % 2 == 0:
                    nc.scalar.copy(u_sb[:], u_ps[:, ts(jc, CT)])
                else:
                    nc.vector.tensor_copy(u_sb[:], u_ps[:, ts(jc, CT)])
                nc.sync.dma_start(out=uout[:, ts(jc, CT)], in_=u_sb[:])

    nc.compile()
    return nc


def make_in_maps(q, r):
    """Host-side sharding: per-core augmented bf16 hi/lo matrices."""
    import ml_dtypes

    bf = ml_dtypes.bfloat16
    q = np.ascontiguousarray(np.asarray(q, np.float32))
    r = np.ascontiguousarray(np.asarray(r, np.float32))
    in_maps = []
    for core in range(NCORES):
        b, h = core // 2, core % 2
        rm = r[b * NB : (b + 1) * NB]
        qm = q[b * NB : (b + 1) * NB]
        rc = (rm - rm.mean(0, keepdims=True)).astype(np.float32)
        hi = rc.astype(bf)
        lo = (rc - hi.astype(np.float32)).astype(bf)
        rr = hi.astype(np.float32) + lo.astype(np.float32)
        n2 = (rr * rr).sum(1).astype(np.float32)
        n2_hi = n2.astype(bf)
        n2_lo = (n2 - n2_hi.astype(np.float32)).astype(bf)

        rbgs = SLOT_RBG[h]
        rowsel = np.concatenate(
            [np.arange(g * 128, (g + 1) * 128) for g in rbgs]
        )
        ones_i = np.ones(RB * 128, bf)
        ones_j = np.ones(NB, bf)
        rowsL, rowsR = [], []
        for ax in range(3):
            rowsL += [hi[rowsel, ax], hi[rowsel, ax], lo[rowsel, ax]]
            rowsR += [
                (-2.0 * hi[:, ax].astype(np.float32)).astype(bf),
                (-2.0 * lo[:, ax].astype(np.float32)).astype(bf),
                (-2.0 * hi[:, ax].astype(np.float32)).astype(bf),
            ]
        rowsL += [n2_hi[rowsel], n2_lo[rowsel], ones_i, ones_i]
        rowsR += [ones_j, ones_j, n2_hi, n2_lo]
        augL_np = np.ascontiguousarray(np.stack(rowsL).astype(bf))
        augR_np = np.ascontiguousarray(np.stack(rowsR).astype(bf))

        qi = qm[rowsel]  # [RB*128, NQ] slot-ordered
        q1_np = np.ascontiguousarray(
            qi.reshape(RB, 128, NQ).transpose(1, 0, 2).reshape(128, RB * NQ)
        ).astype(np.float32)
        q2_np = np.ascontiguousarray(2.0 * q1_np)

        # diag index per chunk: the diagonal sits in chunk 0 of each slot
        # at within-window index 128*(rbg mod 4) + p.
        diag_np = np.full((128, N_CHUNKS), -1.0, np.float32)
        p = np.arange(128, dtype=np.float32)
        cidx = 0
        for t in range(RB):
            g = rbgs[t]
            for ci in range(len(CHUNKS[NJC[t]])):
                if ci == 0:
                    diag_np[:, cidx] = 128.0 * (g % 4) + p
                cidx += 1
        in_maps.append(
            {
                "augL": augL_np,
                "augR": augR_np,
                "q1": q1_np,
                "q2": q2_np,
                "diag": diag_np,
            }
        )
    return in_maps


def reduce_outputs(q, results):
    """Host-side gather: u[8,2048] per core -> pot[B].

    The kernel zeroes the diagonal exactly (select), so the self term
    sum(q^2)/(2*pi)^1.5 is added here in f64.
    """
    q = np.asarray(q, np.float32)
    pots = np.zeros(B, np.float64)
    for core in range(NCORES):
        b = core // 2
        u = results[core]["uout"].astype(np.float64)
        qm = q[b * NB : (b + 1) * NB].astype(np.float64)
        pots[b] += (u * qm.T).sum()
    pots = pots / (4.0 * np.pi)
    for b in range(B):
        qm = q[b * NB : (b + 1) * NB].astype(np.float64)
        pots[b] += (qm**2).sum() / ((2.0 * np.pi) ** 1.5)
    return (pots * NORM_FACTOR).astype(np.float32)


def kernel(q, r, batch):
    global _compiled
    if _compiled is None:
        try:
            _compiled = build_nc(psa_bufs=4)
        except Exception:
            _compiled = build_nc(psa_bufs=2)
    from concourse import bass_utils

    in_maps = make_in_maps(q, r)
    last_err = None
    for attempt in range(3):
        try:
            res = bass_utils.run_bass_kernel_spmd(
                _compiled, in_maps, core_ids=list(range(NCORES))
            )
            return reduce_outputs(q, res.results)
        except Exception as e:  # transient device errors: back off and retry
            last_err = e
            import time

            time.sleep(15 * (attempt + 1))
    raise last_err


# revision 24
# speedup vs baseline: 1.0300x; 1.0172x over previous
"""Trainium2 Bass kernel for the real-space Ewald sum (nn_Ewald).

Math (per molecule b, nb=2048 atoms, 8 charge channels):
    pot_b = sum_{i,j} qq_ij * erf(|rij|/sqrt(2)) / (|rij|+1e-6) / (4*pi)
          + sum_i qq_ii / (2*pi)^1.5            (self term)
    all scaled by NORM_FACTOR.

Kernel formulation per pair tile:
    s_ij  = |ri|^2 + |rj|^2 - 2 ri.rj        (K=13 bf16 hi/lo augmented matmul;
                                              bf16 x bf16 products are exact in
                                              the fp32 PSUM accumulator)
    y     = abs_rsqrt(s)                      (ACT table)
    d     = max(|s| * min(y, 1e4), 1e-4)      (custom DVE op; == sqrt(|s|))
    e     = erf(d / sqrt(2))                  (ACT table)
    w     = (idx != diag) * e * min(y, 1e4)   (custom DVE op; exact 0 diagonal)
    u[c,j] += lam * sum_i q[i,c] w_ij         (PE f32r contraction, PSUM accum;
                                              lam in {1,2} via q vs 2q stationary)
Host: pot_b = sum(u * q^T)/(4*pi) + self term, * NORM.

Symmetry: w is symmetric, so only the block upper triangle is computed.
Row block rb (0..15 within molecule, 128 atoms) covers col windows
jc >= rb//4; the diagonal 512-block gets weight 1 (both orientations of
in-group pairs are computed), strictly-above blocks get weight 2. Each
core takes 8 row blocks whose triangle widths form the multiset
{4,4,3,3,2,2,1,1} so all 8 cores run the identical program (SPMD).

Sharding: 8 cores = 4 molecules x 2 row-block sets.
"""

import numpy as np

B = 4
NB = 2048
NQ = 8
NCORES = 8
RBM = 16            # row blocks per molecule
RB = 8              # row blocks per core
CT = 512            # matmul col tile (PSUM bank)
CAP = 1e4
FLOOR = 1e-4
INV_SQRT2 = 0.7071067811865476
NORM_FACTOR = 90.0474
KA = 13             # augmented contraction depth (bf16 hi/lo split)

# per-slot triangle width in 512-blocks; identical multiset on every core
NJC = [4, 4, 3, 3, 2, 2, 1, 1]
# row blocks (within molecule) per core half
SLOT_RBG = {
    0: [0, 1, 4, 5, 8, 9, 12, 13],
    1: [2, 3, 6, 7, 10, 11, 14, 15],
}
# chunk widths (ACT/DVE op granularity) per NJC: one PSUM bank each
CHUNKS = {n: [CT] * n for n in (1, 2, 3, 4)}
# erf groups: lists of slots per erf call
ERF_GROUPS = [[0], [1], [2, 3], [4, 5, 6, 7]]
# last slot contributing to each u bank (first is always slot 0)
BANK_LAST_SLOT = {0: 1, 1: 3, 2: 5, 3: 7}

# packed Y/D layout: per-slot window offsets
SLOT_W = [n * CT for n in NJC]
SLOT_OFF = np.concatenate([[0], np.cumsum(SLOT_W)]).tolist()
TOTW = SLOT_OFF[-1]  # 10240
N_CHUNKS = sum(len(CHUNKS[n]) for n in NJC)  # 12

_compiled = None
_ops = None


def _register_ops():
    """Register the two custom DVE ops (idempotent)."""
    global _ops
    if _ops is not None:
        return _ops
    from concourse import dve_ops
    from concourse.dve_spec import (
        Spec, Src0, Src1, C0, C1, Zero, maxx, minn, lower, _has_src1,
        Idx, ne,
    )
    from concourse.dve_uop import DveOpSpec

    def mk(name, spec):
        for o in dve_ops.OPS:
            if o.name == name:
                return o
        shas = {}
        for ver in ("v3", "v4"):
            tmp = DveOpSpec(
                name=name,
                opcode=31,
                uops=lower(spec, ver=ver),
                rd1_en=_has_src1(spec),
            )
            shas[ver] = tmp.sha(ver)
        op = dve_ops.DveOp(name, spec, subdim=False, uops_sha=shas)
        dve_ops.OPS.append(op)
        dve_ops._SUB_OPCODE_FOR_NAME[name] = (
            dve_ops._CUSTOM_DVE_ROW_BASE + len(dve_ops.OPS) - 1
        )
        dve_ops.CUSTOM_DVE_SPECS[name] = spec
        return op

    # d = max(|s| * min(y, cap), floor)
    ewald_d = mk(
        "EWALD_D",
        Spec(
            body=maxx(maxx(Src0, Zero - Src0) * minn(Src1, C0), C1),
            reference=lambda in0, in1, s0, s1, imm2: np.maximum(
                np.abs(in0.astype(np.float32))
                * np.minimum(in1.astype(np.float32), np.float32(s0)),
                np.float32(s1),
            ).astype(np.float32),
        ),
    )

    # w = (idx != diag) * e * min(y, cap)
    def _w_ref(in0, in1, s0, s1, imm2):
        in0 = in0.astype(np.float32).reshape(in0.shape[0], -1)
        in1 = in1.astype(np.float32).reshape(in0.shape)
        idx = np.broadcast_to(
            np.arange(in0.shape[1], dtype=np.float32), in0.shape
        )
        dval = np.asarray(s0, np.float32).reshape(-1, 1)
        w = in0 * np.minimum(in1, np.float32(s1))
        return np.where(idx == dval, np.float32(0.0), w).astype(np.float32)

    ewald_w = mk(
        "EWALD_W2",
        Spec(
            body=ne(Idx, C0) * Src0 * minn(Src1, C1),
            reference=_w_ref,
        ),
    )
    _ops = (ewald_d, ewald_w)
    return _ops


def build_nc(psa_bufs=4):
    """Build + compile the per-core Bass program (fixed shapes)."""
    from concourse import bacc, tile
    import concourse.mybir as mybir
    from concourse.bass import ts, ds
    from concourse.tile_rust import add_dep_helper

    ewald_d, ewald_w = _register_ops()
    f32 = mybir.dt.float32
    f32r = mybir.dt.float32r
    bf16 = mybir.dt.bfloat16
    AF = mybir.ActivationFunctionType

    nc = bacc.Bacc(
        "TRN2",
        target_bir_lowering=False,
        debug=False,
        num_devices=NCORES,
    )
    # augL columns are slot-ordered (host maps slots -> molecule row blocks)
    augL = nc.dram_tensor("augL", [KA, RB * 128], bf16, kind="ExternalInput").ap()
    augR = nc.dram_tensor("augR", [KA, NB], bf16, kind="ExternalInput").ap()
    q1 = nc.dram_tensor("q1", [128, RB * NQ], f32r, kind="ExternalInput").ap()
    q2 = nc.dram_tensor("q2", [128, RB * NQ], f32r, kind="ExternalInput").ap()
    diag = nc.dram_tensor("diag", [128, RB], f32, kind="ExternalInput").ap()
    uout = nc.dram_tensor("uout", [NQ, NB], f32, kind="ExternalOutput").ap()

    with tile.TileContext(nc) as tc:
        with (
            tc.tile_pool(name="const", bufs=1) as cpool,
            tc.tile_pool(name="work", bufs=4) as wpool,
            tc.tile_pool(name="big", bufs=1) as bigpool,
            tc.tile_pool(name="psA", bufs=psa_bufs, space="PSUM") as psA,
            tc.tile_pool(name="psU", bufs=1, space="PSUM") as psU,
        ):
            augR_sb = cpool.tile([KA, NB], bf16)
            nc.sync.dma_start(out=augR_sb[:], in_=augR[:])
            augL_sb = cpool.tile([KA, RB * 128], bf16)
            nc.sync.dma_start(out=augL_sb[:], in_=augL[:])
            q1_sb = cpool.tile([128, RB * NQ], f32r)
            nc.gpsimd.dma_start(out=q1_sb[:], in_=q1[:])
            q2_sb = cpool.tile([128, RB * NQ], f32r)
            nc.gpsimd.dma_start(out=q2_sb[:], in_=q2[:])
            diag_sb = cpool.tile([128, RB], f32)
            nc.gpsimd.dma_start(out=diag_sb[:], in_=diag[:])

            Y = bigpool.tile([128, TOTW], f32)
            D = bigpool.tile([128, TOTW], f32)

            # ---- Phase A: s -> y, d  (ACT table: abs_reciprocal_sqrt) ----
            last_rsqrt = None
            chunk_idx = 0
            chunk_of = {}  # (slot, ci) -> global chunk index
            for t in range(RB):
                col0 = NB - NJC[t] * CT  # window start col in molecule
                coff = 0
                for ci, cw in enumerate(CHUNKS[NJC[t]]):
                    chunk_of[(t, ci)] = chunk_idx
                    chunk_idx += 1
                    off = SLOT_OFF[t] + coff
                    s_ps = psA.tile([128, cw], f32, tag="s_ps")
                    for hminor in range(cw // CT):
                        nc.tensor.matmul(
                            s_ps[:, ts(hminor, CT)],
                            augL_sb[:, ts(t, 128)],
                            augR_sb[:, ds(col0 + coff + hminor * CT, CT)],
                            start=True,
                            stop=True,
                        )
                    last_rsqrt = nc.scalar.activation(
                        Y[:, ds(off, cw)], s_ps[:], AF.Abs_reciprocal_sqrt
                    )
                    nc.vector._custom_dve(
                        ewald_d,
                        out=D[:, ds(off, cw)],
                        in0=s_ps[:],
                        in1=Y[:, ds(off, cw)],
                        s0=CAP,
                        s1=FLOOR,
                    )
                    coff += cw

            # ---- Phase B: erf groups (order-pinned after all rsqrts) ----
            u_ps = psU.tile([NQ, 4 * CT], f32)
            for group in ERF_GROUPS:
                goff = SLOT_OFF[group[0]]
                gw = sum(SLOT_W[t] for t in group)
                einst = nc.scalar.activation(
                    D[:, ds(goff, gw)],
                    D[:, ds(goff, gw)],
                    AF.Erf,
                    scale=INV_SQRT2,
                )
                add_dep_helper(
                    einst.ins, last_rsqrt.ins, sync=False,
                    reason="ACT table set order: all rsqrt before any erf",
                )
                for t in group:
                    jc0 = 4 - NJC[t]  # first (diagonal) 512-block of window
                    wt = SLOT_W[t]
                    off = SLOT_OFF[t]
                    w = wpool.tile([128, wt], f32r, tag="w")
                    nc.vector._custom_dve(
                        ewald_w,
                        out=w[:],
                        in0=D[:, ds(off, wt)],
                        in1=Y[:, ds(off, wt)],
                        s0=diag_sb[:, ds(t, 1)],
                        s1=CAP,
                    )
                    for hminor in range(wt // CT):
                        jc = jc0 + hminor
                        qs = q1_sb if jc == jc0 else q2_sb
                        nc.tensor.matmul(
                            u_ps[:, ts(jc, CT)],
                            qs[:, ds(t * NQ, NQ)],
                            w[:, ts(hminor, CT)],
                            start=(t == 0),
                            stop=(t == BANK_LAST_SLOT[jc]),
                        )

            for jc in range(4):
                u_sb = wpool.tile([NQ, CT], f32, tag="u_sb")
                if jc % 2 == 0:
                    nc.scalar.copy(u_sb[:], u_ps[:, ts(jc, CT)])
                else:
                    nc.vector.tensor_copy(u_sb[:], u_ps[:, ts(jc, CT)])
                nc.sync.dma_start(out=uout[:, ts(jc, CT)], in_=u_sb[:])

    nc.compile()
    return nc


def make_in_maps(q, r):
    """Host-side sharding: per-core augmented bf16 hi/lo matrices."""
    import ml_dtypes

    bf = ml_dtypes.bfloat16
    q = np.ascontiguousarray(np.asarray(q, np.float32))
    r = np.ascontiguousarray(np.asarray(r, np.float32))
    in_maps = []
    for core in range(NCORES):
        b, h = core // 2, core % 2
        rm = r[b * NB : (b + 1) * NB]
        qm = q[b * NB : (b + 1) * NB]
        rc = (rm - rm.mean(0, keepdims=True)).astype(np.float32)
        hi = rc.astype(bf)
        lo = (rc - hi.astype(np.float32)).astype(bf)
        rr = hi.astype(np.float32) + lo.astype(np.float32)
        n2 = (rr * rr).sum(1).astype(np.float32)
        n2_hi = n2.astype(bf)
        n2_lo = (n2 - n2_hi.astype(np.float32)).astype(bf)

        rbgs = SLOT_RBG[h]
        rowsel = np.concatenate(
            [np.arange(g * 128, (g + 1) * 128) for g in rbgs]
        )
        ones_i = np.ones(RB * 128, bf)
        ones_j = np.ones(NB, bf)
        rowsL, rowsR = [], []
        for ax in range(3):
            rowsL += [hi[rowsel, ax], hi[rowsel, ax], lo[rowsel, ax]]
            rowsR += [
                (-2.0 * hi[:, ax].astype(np.float32)).astype(bf),
                (-2.0 * lo[:, ax].astype(np.float32)).astype(bf),
                (-2.0 * hi[:, ax].astype(np.float32)).astype(bf),
            ]
        rowsL += [n2_hi[rowsel], n2_lo[rowsel], ones_i, ones_i]
        rowsR += [ones_j, ones_j, n2_hi, n2_lo]
        augL_np = np.ascontiguousarray(np.stack(rowsL).astype(bf))
        augR_np = np.ascontiguousarray(np.stack(rowsR).astype(bf))

        qi = qm[rowsel]  # [RB*128, NQ] slot-ordered
        q1_np = np.ascontiguousarray(
            qi.reshape(RB, 128, NQ).transpose(1, 0, 2).reshape(128, RB * NQ)
        ).astype(np.float32)
        q2_np = np.ascontiguousarray(2.0 * q1_np)

        # diag index per slot: the diagonal sits at within-window index
        # 128*(rbg mod 4) + p (each window starts at its diagonal block).
        diag_np = np.zeros((128, RB), np.float32)
        p = np.arange(128, dtype=np.float32)
        for t in range(RB):
            g = rbgs[t]
            diag_np[:, t] = 128.0 * (g % 4) + p
        in_maps.append(
            {
                "augL": augL_np,
                "augR": augR_np,
                "q1": q1_np,
                "q2": q2_np,
                "diag": diag_np,
            }
        )
    return in_maps


def reduce_outputs(q, results):
    """Host-side gather: u[8,2048] per core -> pot[B].

    The kernel zeroes the diagonal exactly (select), so the self term
    sum(q^2)/(2*pi)^1.5 is added here in f64.
    """
    q = np.asarray(q, np.float32)
    pots = np.zeros(B, np.float64)
    for core in range(NCORES):
        b = core // 2
        u = results[core]["uout"].astype(np.float64)
        qm = q[b * NB : (b + 1) * NB].astype(np.float64)
        pots[b] += (u * qm.T).sum()
    pots = pots / (4.0 * np.pi)
    for b in range(B):
        qm = q[b * NB : (b + 1) * NB].astype(np.float64)
        pots[b] += (qm**2).sum() / ((2.0 * np.pi) ** 1.5)
    return (pots * NORM_FACTOR).astype(np.float32)


def kernel(q, r, batch):
    global _compiled
    if _compiled is None:
        try:
            _compiled = build_nc(psa_bufs=4)
        except Exception:
            _compiled = build_nc(psa_bufs=2)
    from concourse import bass_utils

    in_maps = make_in_maps(q, r)
    last_err = None
    for attempt in range(3):
        try:
            res = bass_utils.run_bass_kernel_spmd(
                _compiled, in_maps, core_ids=list(range(NCORES))
            )
            return reduce_outputs(q, res.results)
        except Exception as e:  # transient device errors: back off and retry
            last_err = e
            import time

            time.sleep(15 * (attempt + 1))
    raise last_err


# revision 26
# speedup vs baseline: 1.0582x; 1.0274x over previous
"""Trainium2 Bass kernel for the real-space Ewald sum (nn_Ewald).

Math (per molecule b, nb=2048 atoms, 8 charge channels):
    pot_b = sum_{i,j} qq_ij * erf(|rij|/sqrt(2)) / (|rij|+1e-6) / (4*pi)
          + sum_i qq_ii / (2*pi)^1.5            (self term)
    all scaled by NORM_FACTOR.

Kernel formulation per pair tile:
    s_ij  = |ri|^2 + |rj|^2 - 2 ri.rj        (K=13 bf16 hi/lo augmented matmul;
                                              bf16 x bf16 products are exact in
                                              the fp32 PSUM accumulator)
    y     = abs_rsqrt(s)                      (ACT table)
    d     = max(|s| * min(y, 1e4), 1e-4)      (custom DVE op; == sqrt(|s|))
    e     = erf(d / sqrt(2))                  (ACT table)
    w     = (idx != diag) * e * min(y, 1e4)   (custom DVE op; exact 0 diagonal)
    u[c,j] += lam * sum_i q[i,c] w_ij         (PE f32r contraction, PSUM accum;
                                              lam in {1,2} via q vs 2q stationary)
Host: pot_b = sum(u * q^T)/(4*pi) + self term, * NORM.

Symmetry: w is symmetric, so only the block upper triangle is computed.
Row block rb (0..15 within molecule, 128 atoms) covers col windows
jc >= rb//4; the diagonal 512-block gets weight 1 (both orientations of
in-group pairs are computed), strictly-above blocks get weight 2. Each
core takes 8 row blocks whose triangle widths form the multiset
{4,4,3,3,2,2,1,1} so all 8 cores run the identical program (SPMD).

Sharding: 8 cores = 4 molecules x 2 row-block sets.
"""

import numpy as np

B = 4
NB = 2048
NQ = 8
NCORES = 8
RBM = 16            # row blocks per molecule
RB = 8              # row blocks per core
CT = 512            # matmul col tile (PSUM bank)
CAP = 1e4
FLOOR = 1e-4
INV_SQRT2 = 0.7071067811865476
NORM_FACTOR = 90.0474
KA = 13             # augmented contraction depth (bf16 hi/lo split)

# per-slot triangle width in 512-blocks; identical multiset on every core
NJC = [4, 4, 3, 3, 2, 2, 1, 1]
# row blocks (within molecule) per core half
SLOT_RBG = {
    0: [0, 1, 4, 5, 8, 9, 12, 13],
    1: [2, 3, 6, 7, 10, 11, 14, 15],
}
# chunk widths (ACT/DVE op granularity) per NJC: one PSUM bank each
CHUNKS = {n: [CT] * n for n in (1, 2, 3, 4)}
# erf groups: lists of slots per erf call
ERF_GROUPS = [[0], [1], [2, 3], [4, 5, 6, 7]]
# last slot contributing to each u bank (first is always slot 0)
BANK_LAST_SLOT = {0: 1, 1: 3, 2: 5, 3: 7}

# packed Y/D layout: per-slot window offsets
SLOT_W = [n * CT for n in NJC]
SLOT_OFF = np.concatenate([[0], np.cumsum(SLOT_W)]).tolist()
TOTW = SLOT_OFF[-1]  # 10240
N_CHUNKS = sum(len(CHUNKS[n]) for n in NJC)  # 12

_compiled = None
_ops = None


def _register_ops():
    """Register the two custom DVE ops (idempotent)."""
    global _ops
    if _ops is not None:
        return _ops
    from concourse import dve_ops
    from concourse.dve_spec import (
        Spec, Src0, Src1, C0, C1, Zero, maxx, minn, lower, _has_src1,
        Idx, ne,
    )
    from concourse.dve_uop import DveOpSpec

    def mk(name, spec):
        for o in dve_ops.OPS:
            if o.name == name:
                return o
        shas = {}
        for ver in ("v3", "v4"):
            tmp = DveOpSpec(
                name=name,
                opcode=31,
                uops=lower(spec, ver=ver),
                rd1_en=_has_src1(spec),
            )
            shas[ver] = tmp.sha(ver)
        op = dve_ops.DveOp(name, spec, subdim=False, uops_sha=shas)
        dve_ops.OPS.append(op)
        dve_ops._SUB_OPCODE_FOR_NAME[name] = (
            dve_ops._CUSTOM_DVE_ROW_BASE + len(dve_ops.OPS) - 1
        )
        dve_ops.CUSTOM_DVE_SPECS[name] = spec
        return op

    # d = max(|s| * min(y, cap), floor)
    ewald_d = mk(
        "EWALD_D",
        Spec(
            body=maxx(maxx(Src0, Zero - Src0) * minn(Src1, C0), C1),
            reference=lambda in0, in1, s0, s1, imm2: np.maximum(
                np.abs(in0.astype(np.float32))
                * np.minimum(in1.astype(np.float32), np.float32(s0)),
                np.float32(s1),
            ).astype(np.float32),
        ),
    )

    # w = (idx != diag) * e * min(y, cap)
    def _w_ref(in0, in1, s0, s1, imm2):
        in0 = in0.astype(np.float32).reshape(in0.shape[0], -1)
        in1 = in1.astype(np.float32).reshape(in0.shape)
        idx = np.broadcast_to(
            np.arange(in0.shape[1], dtype=np.float32), in0.shape
        )
        dval = np.asarray(s0, np.float32).reshape(-1, 1)
        w = in0 * np.minimum(in1, np.float32(s1))
        return np.where(idx == dval, np.float32(0.0), w).astype(np.float32)

    ewald_w = mk(
        "EWALD_W2",
        Spec(
            body=ne(Idx, C0) * Src0 * minn(Src1, C1),
            reference=_w_ref,
        ),
    )
    _ops = (ewald_d, ewald_w)
    return _ops


def build_nc(psa_bufs=8):
    """Build + compile the per-core Bass program (fixed shapes)."""
    from concourse import bacc, tile
    import concourse.mybir as mybir
    from concourse.bass import ts, ds
    from concourse.tile_rust import add_dep_helper

    ewald_d, ewald_w = _register_ops()
    f32 = mybir.dt.float32
    f32r = mybir.dt.float32r
    bf16 = mybir.dt.bfloat16
    AF = mybir.ActivationFunctionType

    nc = bacc.Bacc(
        "TRN2",
        target_bir_lowering=False,
        debug=False,
        num_devices=NCORES,
    )
    # augL columns are slot-ordered (host maps slots -> molecule row blocks)
    augL = nc.dram_tensor("augL", [KA, RB * 128], bf16, kind="ExternalInput").ap()
    augR = nc.dram_tensor("augR", [KA, NB], bf16, kind="ExternalInput").ap()
    q1 = nc.dram_tensor("q1", [128, RB * NQ], f32r, kind="ExternalInput").ap()
    q2 = nc.dram_tensor("q2", [128, RB * NQ], f32r, kind="ExternalInput").ap()
    diag = nc.dram_tensor("diag", [128, RB], f32, kind="ExternalInput").ap()
    uout = nc.dram_tensor("uout", [NQ, NB], f32, kind="ExternalOutput").ap()

    with tile.TileContext(nc) as tc:
        with (
            tc.tile_pool(name="const", bufs=1) as cpool,
            tc.tile_pool(name="work", bufs=4) as wpool,
            tc.tile_pool(name="big", bufs=1) as bigpool,
            tc.tile_pool(name="psA", bufs=psa_bufs, space="PSUM") as psA,
        ):
            augR_sb = cpool.tile([KA, NB], bf16)
            nc.sync.dma_start(out=augR_sb[:], in_=augR[:])
            augL_sb = cpool.tile([KA, RB * 128], bf16)
            nc.sync.dma_start(out=augL_sb[:], in_=augL[:])
            q1_sb = cpool.tile([128, RB * NQ], f32r)
            nc.gpsimd.dma_start(out=q1_sb[:], in_=q1[:])
            q2_sb = cpool.tile([128, RB * NQ], f32r)
            nc.gpsimd.dma_start(out=q2_sb[:], in_=q2[:])
            diag_sb = cpool.tile([128, RB], f32)
            nc.gpsimd.dma_start(out=diag_sb[:], in_=diag[:])

            Y = bigpool.tile([128, TOTW], f32)
            D = bigpool.tile([128, TOTW], f32)

            # ---- Phase A: s -> y, d  (ACT table: abs_reciprocal_sqrt) ----
            last_rsqrt = None
            chunk_idx = 0
            chunk_of = {}  # (slot, ci) -> global chunk index
            for t in range(RB):
                col0 = NB - NJC[t] * CT  # window start col in molecule
                coff = 0
                for ci, cw in enumerate(CHUNKS[NJC[t]]):
                    chunk_of[(t, ci)] = chunk_idx
                    chunk_idx += 1
                    off = SLOT_OFF[t] + coff
                    s_ps = psA.tile([128, cw], f32, tag="s_ps")
                    for hminor in range(cw // CT):
                        nc.tensor.matmul(
                            s_ps[:, ts(hminor, CT)],
                            augL_sb[:, ts(t, 128)],
                            augR_sb[:, ds(col0 + coff + hminor * CT, CT)],
                            start=True,
                            stop=True,
                        )
                    last_rsqrt = nc.scalar.activation(
                        Y[:, ds(off, cw)], s_ps[:], AF.Abs_reciprocal_sqrt
                    )
                    nc.vector._custom_dve(
                        ewald_d,
                        out=D[:, ds(off, cw)],
                        in0=s_ps[:],
                        in1=Y[:, ds(off, cw)],
                        s0=CAP,
                        s1=FLOOR,
                    )
                    coff += cw

            # ---- Phase B: erf groups (order-pinned after all rsqrts) ----
            # u banks share the phase-A pool slots (phase A has drained)
            u_banks = [
                psA.tile([NQ, CT], f32, tag="s_ps", name=f"u_bank{j}")
                for j in range(4)
            ]
            for group in ERF_GROUPS:
                goff = SLOT_OFF[group[0]]
                gw = sum(SLOT_W[t] for t in group)
                einst = nc.scalar.activation(
                    D[:, ds(goff, gw)],
                    D[:, ds(goff, gw)],
                    AF.Erf,
                    scale=INV_SQRT2,
                )
                add_dep_helper(
                    einst.ins, last_rsqrt.ins, sync=False,
                    reason="ACT table set order: all rsqrt before any erf",
                )
                for t in group:
                    jc0 = 4 - NJC[t]  # first (diagonal) 512-block of window
                    wt = SLOT_W[t]
                    off = SLOT_OFF[t]
                    w = wpool.tile([128, wt], f32r, tag="w")
                    nc.vector._custom_dve(
                        ewald_w,
                        out=w[:],
                        in0=D[:, ds(off, wt)],
                        in1=Y[:, ds(off, wt)],
                        s0=diag_sb[:, ds(t, 1)],
                        s1=CAP,
                    )
                    for hminor in range(wt // CT):
                        jc = jc0 + hminor
                        qs = q1_sb if jc == jc0 else q2_sb
                        nc.tensor.matmul(
                            u_banks[jc][:],
                            qs[:, ds(t * NQ, NQ)],
                            w[:, ts(hminor, CT)],
                            start=(t == 0),
                            stop=(t == BANK_LAST_SLOT[jc]),
                        )

            for jc in range(4):
                u_sb = wpool.tile([NQ, CT], f32, tag="u_sb")
                if jc in (2, 3):
                    nc.scalar.copy(u_sb[:], u_banks[jc][:])
                else:
                    nc.vector.tensor_copy(u_sb[:], u_banks[jc][:])
                nc.sync.dma_start(out=uout[:, ts(jc, CT)], in_=u_sb[:])

    nc.compile()
    return nc


def make_in_maps(q, r):
    """Host-side sharding: per-core augmented bf16 hi/lo matrices."""
    import ml_dtypes

    bf = ml_dtypes.bfloat16
    q = np.ascontiguousarray(np.asarray(q, np.float32))
    r = np.ascontiguousarray(np.asarray(r, np.float32))
    in_maps = []
    for core in range(NCORES):
        b, h = core // 2, core % 2
        rm = r[b * NB : (b + 1) * NB]
        qm = q[b * NB : (b + 1) * NB]
        rc = (rm - rm.mean(0, keepdims=True)).astype(np.float32)
        hi = rc.astype(bf)
        lo = (rc - hi.astype(np.float32)).astype(bf)
        rr = hi.astype(np.float32) + lo.astype(np.float32)
        n2 = (rr * rr).sum(1).astype(np.float32)
        n2_hi = n2.astype(bf)
        n2_lo = (n2 - n2_hi.astype(np.float32)).astype(bf)

        rbgs = SLOT_RBG[h]
        rowsel = np.concatenate(
            [np.arange(g * 128, (g + 1) * 128) for g in rbgs]
        )
        ones_i = np.ones(RB * 128, bf)
        ones_j = np.ones(NB, bf)
        rowsL, rowsR = [], []
        for ax in range(3):
            rowsL += [hi[rowsel, ax], hi[rowsel, ax], lo[rowsel, ax]]
            rowsR += [
                (-2.0 * hi[:, ax].astype(np.float32)).astype(bf),
                (-2.0 * lo[:, ax].astype(np.float32)).astype(bf),
                (-2.0 * hi[:, ax].astype(np.float32)).astype(bf),
            ]
        rowsL += [n2_hi[rowsel], n2_lo[rowsel], ones_i, ones_i]
        rowsR += [ones_j, ones_j, n2_hi, n2_lo]
        augL_np = np.ascontiguousarray(np.stack(rowsL).astype(bf))
        augR_np = np.ascontiguousarray(np.stack(rowsR).astype(bf))

        qi = qm[rowsel]  # [RB*128, NQ] slot-ordered
        q1_np = np.ascontiguousarray(
            qi.reshape(RB, 128, NQ).transpose(1, 0, 2).reshape(128, RB * NQ)
        ).astype(np.float32)
        q2_np = np.ascontiguousarray(2.0 * q1_np)

        # diag index per slot: the diagonal sits at within-window index
        # 128*(rbg mod 4) + p (each window starts at its diagonal block).
        diag_np = np.zeros((128, RB), np.float32)
        p = np.arange(128, dtype=np.float32)
        for t in range(RB):
            g = rbgs[t]
            diag_np[:, t] = 128.0 * (g % 4) + p
        in_maps.append(
            {
                "augL": augL_np,
                "augR": augR_np,
                "q1": q1_np,
                "q2": q2_np,
                "diag": diag_np,
            }
        )
    return in_maps


def reduce_outputs(q, results):
    """Host-side gather: u[8,2048] per core -> pot[B].

    The kernel zeroes the diagonal exactly (select), so the self term
    sum(q^2)/(2*pi)^1.5 is added here in f64.
    """
    q = np.asarray(q, np.float32)
    pots = np.zeros(B, np.float64)
    for core in range(NCORES):
        b = core // 2
        u = results[core]["uout"].astype(np.float64)
        qm = q[b * NB : (b + 1) * NB].astype(np.float64)
        pots[b] += (u * qm.T).sum()
    pots = pots / (4.0 * np.pi)
    for b in range(B):
        qm = q[b * NB : (b + 1) * NB].astype(np.float64)
        pots[b] += (qm**2).sum() / ((2.0 * np.pi) ** 1.5)
    return (pots * NORM_FACTOR).astype(np.float32)


def kernel(q, r, batch):
    global _compiled
    if _compiled is None:
        try:
            _compiled = build_nc(psa_bufs=8)
        except Exception:
            _compiled = build_nc(psa_bufs=4)
    from concourse import bass_utils

    in_maps = make_in_maps(q, r)
    last_err = None
    for attempt in range(3):
        try:
            res = bass_utils.run_bass_kernel_spmd(
                _compiled, in_maps, core_ids=list(range(NCORES))
            )
            return reduce_outputs(q, res.results)
        except Exception as e:  # transient device errors: back off and retry
            last_err = e
            import time

            time.sleep(15 * (attempt + 1))
    raise last_err


# revision 27
# speedup vs baseline: 1.0793x; 1.0199x over previous
"""Trainium2 Bass kernel for the real-space Ewald sum (nn_Ewald).

Math (per molecule b, nb=2048 atoms, 8 charge channels):
    pot_b = sum_{i,j} qq_ij * erf(|rij|/sqrt(2)) / (|rij|+1e-6) / (4*pi)
          + sum_i qq_ii / (2*pi)^1.5            (self term)
    all scaled by NORM_FACTOR.

Kernel formulation per pair tile:
    s_ij  = |ri|^2 + |rj|^2 - 2 ri.rj        (K=13 bf16 hi/lo augmented matmul;
                                              bf16 x bf16 products are exact in
                                              the fp32 PSUM accumulator)
    y     = abs_rsqrt(s)                      (ACT table)
    d     = max(|s| * min(y, 1e4), 1e-4)      (custom DVE op; == sqrt(|s|))
    e     = erf(d / sqrt(2))                  (ACT table)
    w     = (idx != diag) * e * min(y, 1e4)   (custom DVE op; exact 0 diagonal)
    u[c,j] += lam * sum_i q[i,c] w_ij         (PE f32r contraction, PSUM accum;
                                              lam in {1,2} via q vs 2q stationary)
Host: pot_b = sum(u * q^T)/(4*pi) + self term, * NORM.

Symmetry: w is symmetric, so only the block upper triangle is computed.
Row block rb (0..15 within molecule, 128 atoms) covers col windows
jc >= rb//4; the diagonal 512-block gets weight 1 (both orientations of
in-group pairs are computed), strictly-above blocks get weight 2. Each
core takes 8 row blocks whose triangle widths form the multiset
{4,4,3,3,2,2,1,1} so all 8 cores run the identical program (SPMD).

Sharding: 8 cores = 4 molecules x 2 row-block sets.
"""

import numpy as np

B = 4
NB = 2048
NQ = 8
NCORES = 8
RBM = 16            # row blocks per molecule
RB = 8              # row blocks per core
CT = 512            # matmul col tile (PSUM bank)
CAP = 1e4
FLOOR = 1e-4
INV_SQRT2 = 0.7071067811865476
NORM_FACTOR = 90.0474
KA = 13             # augmented contraction depth (bf16 hi/lo split)

# per-slot triangle width in 512-blocks; identical multiset on every core
NJC = [4, 4, 3, 3, 2, 2, 1, 1]
# row blocks (within molecule) per core half
SLOT_RBG = {
    0: [0, 1, 4, 5, 8, 9, 12, 13],
    1: [2, 3, 6, 7, 10, 11, 14, 15],
}
# chunk widths (ACT/DVE op granularity) per NJC: one PSUM bank each
CHUNKS = {n: [CT] * n for n in (1, 2, 3, 4)}
# erf groups: lists of (slot, offset-in-window, width) per erf call;
# slot 0 is split so phase B starts after a 1024-wide erf
ERF_GROUPS = [
    [(0, 0, 1024)],
    [(0, 1024, 1024)],
    [(1, 0, 2048)],
    [(2, 0, 1536), (3, 0, 1536)],
    [(4, 0, 1024), (5, 0, 1024), (6, 0, 512), (7, 0, 512)],
]
# last slot contributing to each u bank (first is always slot 0)
BANK_LAST_SLOT = {0: 1, 1: 3, 2: 5, 3: 7}

# packed Y/D layout: per-slot window offsets
SLOT_W = [n * CT for n in NJC]
SLOT_OFF = np.concatenate([[0], np.cumsum(SLOT_W)]).tolist()
TOTW = SLOT_OFF[-1]  # 10240
N_CHUNKS = sum(len(CHUNKS[n]) for n in NJC)  # 12

_compiled = None
_ops = None


def _register_ops():
    """Register the two custom DVE ops (idempotent)."""
    global _ops
    if _ops is not None:
        return _ops
    from concourse import dve_ops
    from concourse.dve_spec import (
        Spec, Src0, Src1, C0, C1, Zero, maxx, minn, lower, _has_src1,
        Idx, ne,
    )
    from concourse.dve_uop import DveOpSpec

    def mk(name, spec):
        for o in dve_ops.OPS:
            if o.name == name:
                return o
        shas = {}
        for ver in ("v3", "v4"):
            tmp = DveOpSpec(
                name=name,
                opcode=31,
                uops=lower(spec, ver=ver),
                rd1_en=_has_src1(spec),
            )
            shas[ver] = tmp.sha(ver)
        op = dve_ops.DveOp(name, spec, subdim=False, uops_sha=shas)
        dve_ops.OPS.append(op)
        dve_ops._SUB_OPCODE_FOR_NAME[name] = (
            dve_ops._CUSTOM_DVE_ROW_BASE + len(dve_ops.OPS) - 1
        )
        dve_ops.CUSTOM_DVE_SPECS[name] = spec
        return op

    # d = max(|s| * min(y, cap), floor)
    ewald_d = mk(
        "EWALD_D",
        Spec(
            body=maxx(maxx(Src0, Zero - Src0) * minn(Src1, C0), C1),
            reference=lambda in0, in1, s0, s1, imm2: np.maximum(
                np.abs(in0.astype(np.float32))
                * np.minimum(in1.astype(np.float32), np.float32(s0)),
                np.float32(s1),
            ).astype(np.float32),
        ),
    )

    # w = (idx != diag) * e * min(y, cap)
    def _w_ref(in0, in1, s0, s1, imm2):
        in0 = in0.astype(np.float32).reshape(in0.shape[0], -1)
        in1 = in1.astype(np.float32).reshape(in0.shape)
        idx = np.broadcast_to(
            np.arange(in0.shape[1], dtype=np.float32), in0.shape
        )
        dval = np.asarray(s0, np.float32).reshape(-1, 1)
        w = in0 * np.minimum(in1, np.float32(s1))
        return np.where(idx == dval, np.float32(0.0), w).astype(np.float32)

    ewald_w = mk(
        "EWALD_W2",
        Spec(
            body=ne(Idx, C0) * Src0 * minn(Src1, C1),
            reference=_w_ref,
        ),
    )
    _ops = (ewald_d, ewald_w)
    return _ops


def build_nc(psa_bufs=8):
    """Build + compile the per-core Bass program (fixed shapes)."""
    from concourse import bacc, tile
    import concourse.mybir as mybir
    from concourse.bass import ts, ds
    from concourse.tile_rust import add_dep_helper

    ewald_d, ewald_w = _register_ops()
    f32 = mybir.dt.float32
    f32r = mybir.dt.float32r
    bf16 = mybir.dt.bfloat16
    AF = mybir.ActivationFunctionType

    nc = bacc.Bacc(
        "TRN2",
        target_bir_lowering=False,
        debug=False,
        num_devices=NCORES,
    )
    # aug columns: slot-ordered stationary block, then the full col block
    augc = nc.dram_tensor(
        "augc", [KA, RB * 128 + NB], bf16, kind="ExternalInput"
    ).ap()
    q1 = nc.dram_tensor("q1", [128, RB * NQ], f32r, kind="ExternalInput").ap()
    q2 = nc.dram_tensor("q2", [128, RB * NQ], f32r, kind="ExternalInput").ap()
    diag = nc.dram_tensor("diag", [128, RB], f32, kind="ExternalInput").ap()
    uout = nc.dram_tensor("uout", [NQ, NB], f32, kind="ExternalOutput").ap()

    with tile.TileContext(nc) as tc:
        with (
            tc.tile_pool(name="const", bufs=1) as cpool,
            tc.tile_pool(name="work", bufs=4) as wpool,
            tc.tile_pool(name="big", bufs=1) as bigpool,
            tc.tile_pool(name="psA", bufs=psa_bufs, space="PSUM") as psA,
        ):
            augc_sb = cpool.tile([KA, RB * 128 + NB], bf16)
            nc.sync.dma_start(out=augc_sb[:], in_=augc[:])
            q1_sb = cpool.tile([128, RB * NQ], f32r)
            nc.gpsimd.dma_start(out=q1_sb[:], in_=q1[:])
            q2_sb = cpool.tile([128, RB * NQ], f32r)
            nc.gpsimd.dma_start(out=q2_sb[:], in_=q2[:])
            diag_sb = cpool.tile([128, RB], f32)
            nc.gpsimd.dma_start(out=diag_sb[:], in_=diag[:])

            Y = bigpool.tile([128, TOTW], f32)
            D = bigpool.tile([128, TOTW], f32)

            # ---- Phase A: s -> y, d  (ACT table: abs_reciprocal_sqrt) ----
            last_rsqrt = None
            chunk_idx = 0
            chunk_of = {}  # (slot, ci) -> global chunk index
            for t in range(RB):
                col0 = NB - NJC[t] * CT  # window start col in molecule
                coff = 0
                for ci, cw in enumerate(CHUNKS[NJC[t]]):
                    chunk_of[(t, ci)] = chunk_idx
                    chunk_idx += 1
                    off = SLOT_OFF[t] + coff
                    s_ps = psA.tile([128, cw], f32, tag="s_ps")
                    for hminor in range(cw // CT):
                        nc.tensor.matmul(
                            s_ps[:, ts(hminor, CT)],
                            augc_sb[:, ts(t, 128)],
                            augc_sb[:, ds(RB * 128 + col0 + coff + hminor * CT, CT)],
                            start=True,
                            stop=True,
                        )
                    last_rsqrt = nc.scalar.activation(
                        Y[:, ds(off, cw)], s_ps[:], AF.Abs_reciprocal_sqrt
                    )
                    nc.vector._custom_dve(
                        ewald_d,
                        out=D[:, ds(off, cw)],
                        in0=s_ps[:],
                        in1=Y[:, ds(off, cw)],
                        s0=CAP,
                        s1=FLOOR,
                    )
                    coff += cw

            # ---- Phase B: erf groups (order-pinned after all rsqrts) ----
            # u banks share the phase-A pool slots (phase A has drained)
            u_banks = [
                psA.tile([NQ, CT], f32, tag="s_ps", name=f"u_bank{j}")
                for j in range(4)
            ]
            for group in ERF_GROUPS:
                goff = SLOT_OFF[group[0][0]] + group[0][1]
                gw = sum(pw for (_, _, pw) in group)
                einst = nc.scalar.activation(
                    D[:, ds(goff, gw)],
                    D[:, ds(goff, gw)],
                    AF.Erf,
                    scale=INV_SQRT2,
                )
                add_dep_helper(
                    einst.ins, last_rsqrt.ins, sync=False,
                    reason="ACT table set order: all rsqrt before any erf",
                )
                for (t, poff, pw) in group:
                    jc0 = 4 - NJC[t]  # first (diagonal) 512-block of window
                    off = SLOT_OFF[t] + poff
                    w = wpool.tile([128, pw], f32r, tag="w")
                    nc.vector._custom_dve(
                        ewald_w,
                        out=w[:],
                        in0=D[:, ds(off, pw)],
                        in1=Y[:, ds(off, pw)],
                        s0=diag_sb[:, ds(t, 1)] if poff == 0 else -1.0,
                        s1=CAP,
                    )
                    for hminor in range(pw // CT):
                        jc = jc0 + (poff // CT) + hminor
                        qs = q1_sb if jc == jc0 else q2_sb
                        nc.tensor.matmul(
                            u_banks[jc][:],
                            qs[:, ds(t * NQ, NQ)],
                            w[:, ts(hminor, CT)],
                            start=(t == 0),
                            stop=(t == BANK_LAST_SLOT[jc]),
                        )

            for jc in range(4):
                u_sb = wpool.tile([NQ, CT], f32, tag="u_sb")
                nc.scalar.copy(u_sb[:], u_banks[jc][:])
                eng = nc.sync if jc < 2 else nc.gpsimd
                eng.dma_start(out=uout[:, ts(jc, CT)], in_=u_sb[:])

    nc.compile()
    return nc


def make_in_maps(q, r):
    """Host-side sharding: per-core augmented bf16 hi/lo matrices."""
    import ml_dtypes

    bf = ml_dtypes.bfloat16
    q = np.ascontiguousarray(np.asarray(q, np.float32))
    r = np.ascontiguousarray(np.asarray(r, np.float32))
    in_maps = []
    for core in range(NCORES):
        b, h = core // 2, core % 2
        rm = r[b * NB : (b + 1) * NB]
        qm = q[b * NB : (b + 1) * NB]
        rc = (rm - rm.mean(0, keepdims=True)).astype(np.float32)
        hi = rc.astype(bf)
        lo = (rc - hi.astype(np.float32)).astype(bf)
        rr = hi.astype(np.float32) + lo.astype(np.float32)
        n2 = (rr * rr).sum(1).astype(np.float32)
        n2_hi = n2.astype(bf)
        n2_lo = (n2 - n2_hi.astype(np.float32)).astype(bf)

        rbgs = SLOT_RBG[h]
        rowsel = np.concatenate(
            [np.arange(g * 128, (g + 1) * 128) for g in rbgs]
        )
        ones_i = np.ones(RB * 128, bf)
        ones_j = np.ones(NB, bf)
        rowsL, rowsR = [], []
        for ax in range(3):
            rowsL += [hi[rowsel, ax], hi[rowsel, ax], lo[rowsel, ax]]
            rowsR += [
                (-2.0 * hi[:, ax].astype(np.float32)).astype(bf),
                (-2.0 * lo[:, ax].astype(np.float32)).astype(bf),
                (-2.0 * hi[:, ax].astype(np.float32)).astype(bf),
            ]
        rowsL += [n2_hi[rowsel], n2_lo[rowsel], ones_i, ones_i]
        rowsR += [ones_j, ones_j, n2_hi, n2_lo]
        augc_np = np.ascontiguousarray(
            np.concatenate(
                [np.stack(rowsL).astype(bf), np.stack(rowsR).astype(bf)], axis=1
            )
        )

        qi = qm[rowsel]  # [RB*128, NQ] slot-ordered
        q1_np = np.ascontiguousarray(
            qi.reshape(RB, 128, NQ).transpose(1, 0, 2).reshape(128, RB * NQ)
        ).astype(np.float32)
        q2_np = np.ascontiguousarray(2.0 * q1_np)

        # diag index per slot: the diagonal sits at within-window index
        # 128*(rbg mod 4) + p (each window starts at its diagonal block).
        diag_np = np.zeros((128, RB), np.float32)
        p = np.arange(128, dtype=np.float32)
        for t in range(RB):
            g = rbgs[t]
            diag_np[:, t] = 128.0 * (g % 4) + p
        in_maps.append(
            {"augc": augc_np, "q1": q1_np, "q2": q2_np, "diag": diag_np}
        )
    return in_maps


def reduce_outputs(q, results):
    """Host-side gather: u[8,2048] per core -> pot[B].

    The kernel zeroes the diagonal exactly (select), so the self term
    sum(q^2)/(2*pi)^1.5 is added here in f64.
    """
    q = np.asarray(q, np.float32)
    pots = np.zeros(B, np.float64)
    for core in range(NCORES):
        b = core // 2
        u = results[core]["uout"].astype(np.float64)
        qm = q[b * NB : (b + 1) * NB].astype(np.float64)
        pots[b] += (u * qm.T).sum()
    pots = pots / (4.0 * np.pi)
    for b in range(B):
        qm = q[b * NB : (b + 1) * NB].astype(np.float64)
        pots[b] += (qm**2).sum() / ((2.0 * np.pi) ** 1.5)
    return (pots * NORM_FACTOR).astype(np.float32)


def kernel(q, r, batch):
    global _compiled
    if _compiled is None:
        try:
            _compiled = build_nc(psa_bufs=8)
        except Exception:
            _compiled = build_nc(psa_bufs=4)
    from concourse import bass_utils

    in_maps = make_in_maps(q, r)
    last_err = None
    for attempt in range(3):
        try:
            res = bass_utils.run_bass_kernel_spmd(
                _compiled, in_maps, core_ids=list(range(NCORES))
            )
            return reduce_outputs(q, res.results)
        except Exception as e:  # transient device errors: back off and retry
            last_err = e
            import time

            time.sleep(15 * (attempt + 1))
    raise last_err


# revision 28
# speedup vs baseline: 1.0929x; 1.0126x over previous
"""Trainium2 Bass kernel for the real-space Ewald sum (nn_Ewald).

Math (per molecule b, nb=2048 atoms, 8 charge channels):
    pot_b = sum_{i,j} qq_ij * erf(|rij|/sqrt(2)) / (|rij|+1e-6) / (4*pi)
          + sum_i qq_ii / (2*pi)^1.5            (self term)
    all scaled by NORM_FACTOR.

Kernel formulation per pair tile:
    s_ij  = |ri|^2 + |rj|^2 - 2 ri.rj        (K=13 bf16 hi/lo augmented matmul;
                                              bf16 x bf16 products are exact in
                                              the fp32 PSUM accumulator)
    y     = abs_rsqrt(s)                      (ACT table)
    d     = max(|s| * min(y, 1e4), 1e-4)      (custom DVE op; == sqrt(|s|))
    e     = erf(d / sqrt(2))                  (ACT table)
    w     = (idx != diag) * e * min(y, 1e4)   (custom DVE op; exact 0 diagonal)
    u[c,j] += lam * sum_i q[i,c] w_ij         (PE f32r contraction, PSUM accum;
                                              lam in {1,2} via q vs 2q stationary)
Host: pot_b = sum(u * q^T)/(4*pi) + self term, * NORM.

Symmetry: w is symmetric, so only the block upper triangle is computed.
Row block rb (0..15 within molecule, 128 atoms) covers col windows
jc >= rb//4; the diagonal 512-block gets weight 1 (both orientations of
in-group pairs are computed), strictly-above blocks get weight 2. Each
core takes 8 row blocks whose triangle widths form the multiset
{4,4,3,3,2,2,1,1} so all 8 cores run the identical program (SPMD).

Sharding: 8 cores = 4 molecules x 2 row-block sets.
"""

import numpy as np

B = 4
NB = 2048
NQ = 8
NCORES = 8
RBM = 16            # row blocks per molecule
RB = 8              # row blocks per core
CT = 512            # matmul col tile (PSUM bank)
CAP = 1e4
FLOOR = 1e-4
INV_SQRT2 = 0.7071067811865476
NORM_FACTOR = 90.0474
KA = 13             # augmented contraction depth (bf16 hi/lo split)

# per-slot triangle width in 512-blocks; identical multiset on every core
NJC = [4, 4, 3, 3, 2, 2, 1, 1]
# row blocks (within molecule) per core half
SLOT_RBG = {
    0: [0, 1, 4, 5, 8, 9, 12, 13],
    1: [2, 3, 6, 7, 10, 11, 14, 15],
}
# phase-A chunk widths (PSUM tile / ACT / DVE op granularity)
CHUNKS = {4: [1024, 1024], 3: [1024, 512], 2: [1024], 1: [512]}
# erf groups: lists of (slot, offset-in-window, width) per erf call;
# slot 0 is split so phase B starts after a 1024-wide erf
ERF_GROUPS = [
    [(0, 0, 1024)],
    [(0, 1024, 1024)],
    [(1, 0, 2048)],
    [(2, 0, 1536), (3, 0, 1536)],
    [(4, 0, 1024), (5, 0, 1024), (6, 0, 512), (7, 0, 512)],
]
# last slot contributing to each u bank (first is always slot 0)
BANK_LAST_SLOT = {0: 1, 1: 3, 2: 5, 3: 7}

# packed Y/D layout: per-slot window offsets
SLOT_W = [n * CT for n in NJC]
SLOT_OFF = np.concatenate([[0], np.cumsum(SLOT_W)]).tolist()
TOTW = SLOT_OFF[-1]  # 10240
N_CHUNKS = sum(len(CHUNKS[n]) for n in NJC)  # 12

_compiled = None
_ops = None


def _register_ops():
    """Register the two custom DVE ops (idempotent)."""
    global _ops
    if _ops is not None:
        return _ops
    from concourse import dve_ops
    from concourse.dve_spec import (
        Spec, Src0, Src1, C0, C1, Zero, maxx, minn, lower, _has_src1,
        Idx, ne,
    )
    from concourse.dve_uop import DveOpSpec

    def mk(name, spec):
        for o in dve_ops.OPS:
            if o.name == name:
                return o
        shas = {}
        for ver in ("v3", "v4"):
            tmp = DveOpSpec(
                name=name,
                opcode=31,
                uops=lower(spec, ver=ver),
                rd1_en=_has_src1(spec),
            )
            shas[ver] = tmp.sha(ver)
        op = dve_ops.DveOp(name, spec, subdim=False, uops_sha=shas)
        dve_ops.OPS.append(op)
        dve_ops._SUB_OPCODE_FOR_NAME[name] = (
            dve_ops._CUSTOM_DVE_ROW_BASE + len(dve_ops.OPS) - 1
        )
        dve_ops.CUSTOM_DVE_SPECS[name] = spec
        return op

    # d = max(|s| * min(y, cap), floor)
    ewald_d = mk(
        "EWALD_D",
        Spec(
            body=maxx(maxx(Src0, Zero - Src0) * minn(Src1, C0), C1),
            reference=lambda in0, in1, s0, s1, imm2: np.maximum(
                np.abs(in0.astype(np.float32))
                * np.minimum(in1.astype(np.float32), np.float32(s0)),
                np.float32(s1),
            ).astype(np.float32),
        ),
    )

    # w = (idx != diag) * e * min(y, cap)
    def _w_ref(in0, in1, s0, s1, imm2):
        in0 = in0.astype(np.float32).reshape(in0.shape[0], -1)
        in1 = in1.astype(np.float32).reshape(in0.shape)
        idx = np.broadcast_to(
            np.arange(in0.shape[1], dtype=np.float32), in0.shape
        )
        dval = np.asarray(s0, np.float32).reshape(-1, 1)
        w = in0 * np.minimum(in1, np.float32(s1))
        return np.where(idx == dval, np.float32(0.0), w).astype(np.float32)

    ewald_w = mk(
        "EWALD_W2",
        Spec(
            body=ne(Idx, C0) * Src0 * minn(Src1, C1),
            reference=_w_ref,
        ),
    )
    _ops = (ewald_d, ewald_w)
    return _ops


def build_nc(psa_bufs=4):
    """Build + compile the per-core Bass program (fixed shapes)."""
    from concourse import bacc, tile
    import concourse.mybir as mybir
    from concourse.bass import ts, ds
    from concourse.tile_rust import add_dep_helper

    ewald_d, ewald_w = _register_ops()
    f32 = mybir.dt.float32
    f32r = mybir.dt.float32r
    bf16 = mybir.dt.bfloat16
    AF = mybir.ActivationFunctionType

    nc = bacc.Bacc(
        "TRN2",
        target_bir_lowering=False,
        debug=False,
        num_devices=NCORES,
    )
    # aug columns: slot-ordered stationary block, then the full col block
    augc = nc.dram_tensor(
        "augc", [KA, RB * 128 + NB], bf16, kind="ExternalInput"
    ).ap()
    q1 = nc.dram_tensor("q1", [128, RB * NQ], f32r, kind="ExternalInput").ap()
    q2 = nc.dram_tensor("q2", [128, RB * NQ], f32r, kind="ExternalInput").ap()
    diag = nc.dram_tensor("diag", [128, RB], f32, kind="ExternalInput").ap()
    uout = nc.dram_tensor("uout", [NQ, NB], f32, kind="ExternalOutput").ap()

    with tile.TileContext(nc) as tc:
        with (
            tc.tile_pool(name="const", bufs=1) as cpool,
            tc.tile_pool(name="work", bufs=4) as wpool,
            tc.tile_pool(name="big", bufs=1) as bigpool,
            tc.tile_pool(name="psA", bufs=psa_bufs, space="PSUM") as psA,
        ):
            augc_sb = cpool.tile([KA, RB * 128 + NB], bf16)
            nc.sync.dma_start(out=augc_sb[:], in_=augc[:])
            q1_sb = cpool.tile([128, RB * NQ], f32r)
            nc.gpsimd.dma_start(out=q1_sb[:], in_=q1[:])
            q2_sb = cpool.tile([128, RB * NQ], f32r)
            nc.gpsimd.dma_start(out=q2_sb[:], in_=q2[:])
            diag_sb = cpool.tile([128, RB], f32)
            nc.gpsimd.dma_start(out=diag_sb[:], in_=diag[:])

            Y = bigpool.tile([128, TOTW], f32)
            D = bigpool.tile([128, TOTW], f32)

            # ---- Phase A: s -> y, d  (ACT table: abs_reciprocal_sqrt) ----
            last_rsqrt = None
            chunk_idx = 0
            chunk_of = {}  # (slot, ci) -> global chunk index
            for t in range(RB):
                col0 = NB - NJC[t] * CT  # window start col in molecule
                coff = 0
                for ci, cw in enumerate(CHUNKS[NJC[t]]):
                    chunk_of[(t, ci)] = chunk_idx
                    chunk_idx += 1
                    off = SLOT_OFF[t] + coff
                    s_ps = psA.tile([128, cw], f32, tag="s_ps")
                    for hminor in range(cw // CT):
                        nc.tensor.matmul(
                            s_ps[:, ts(hminor, CT)],
                            augc_sb[:, ts(t, 128)],
                            augc_sb[:, ds(RB * 128 + col0 + coff + hminor * CT, CT)],
                            start=True,
                            stop=True,
                        )
                    last_rsqrt = nc.scalar.activation(
                        Y[:, ds(off, cw)], s_ps[:], AF.Abs_reciprocal_sqrt
                    )
                    nc.vector._custom_dve(
                        ewald_d,
                        out=D[:, ds(off, cw)],
                        in0=s_ps[:],
                        in1=Y[:, ds(off, cw)],
                        s0=CAP,
                        s1=FLOOR,
                    )
                    coff += cw

            # ---- Phase B: erf groups (order-pinned after all rsqrts) ----
            # u banks share the phase-A pool slots (phase A has drained)
            u_banks = [
                psA.tile([NQ, CT], f32, tag="s_ps", name=f"u_bank{j}")
                for j in range(4)
            ]
            for group in ERF_GROUPS:
                goff = SLOT_OFF[group[0][0]] + group[0][1]
                gw = sum(pw for (_, _, pw) in group)
                einst = nc.scalar.activation(
                    D[:, ds(goff, gw)],
                    D[:, ds(goff, gw)],
                    AF.Erf,
                    scale=INV_SQRT2,
                )
                add_dep_helper(
                    einst.ins, last_rsqrt.ins, sync=False,
                    reason="ACT table set order: all rsqrt before any erf",
                )
                for (t, poff, pw) in group:
                    jc0 = 4 - NJC[t]  # first (diagonal) 512-block of window
                    off = SLOT_OFF[t] + poff
                    w = wpool.tile([128, pw], f32r, tag="w")
                    nc.vector._custom_dve(
                        ewald_w,
                        out=w[:],
                        in0=D[:, ds(off, pw)],
                        in1=Y[:, ds(off, pw)],
                        s0=diag_sb[:, ds(t, 1)] if poff == 0 else -1.0,
                        s1=CAP,
                    )
                    for hminor in range(pw // CT):
                        jc = jc0 + (poff // CT) + hminor
                        qs = q1_sb if jc == jc0 else q2_sb
                        nc.tensor.matmul(
                            u_banks[jc][:],
                            qs[:, ds(t * NQ, NQ)],
                            w[:, ts(hminor, CT)],
                            start=(t == 0),
                            stop=(t == BANK_LAST_SLOT[jc]),
                        )

            for jc in range(4):
                u_sb = wpool.tile([NQ, CT], f32, tag="u_sb")
                nc.scalar.copy(u_sb[:], u_banks[jc][:])
                eng = nc.sync if jc < 2 else nc.gpsimd
                eng.dma_start(out=uout[:, ts(jc, CT)], in_=u_sb[:])

    nc.compile()
    return nc


def make_in_maps(q, r):
    """Host-side sharding: per-core augmented bf16 hi/lo matrices."""
    import ml_dtypes

    bf = ml_dtypes.bfloat16
    q = np.ascontiguousarray(np.asarray(q, np.float32))
    r = np.ascontiguousarray(np.asarray(r, np.float32))
    in_maps = []
    for core in range(NCORES):
        b, h = core // 2, core % 2
        rm = r[b * NB : (b + 1) * NB]
        qm = q[b * NB : (b + 1) * NB]
        rc = (rm - rm.mean(0, keepdims=True)).astype(np.float32)
        hi = rc.astype(bf)
        lo = (rc - hi.astype(np.float32)).astype(bf)
        rr = hi.astype(np.float32) + lo.astype(np.float32)
        n2 = (rr * rr).sum(1).astype(np.float32)
        n2_hi = n2.astype(bf)
        n2_lo = (n2 - n2_hi.astype(np.float32)).astype(bf)

        rbgs = SLOT_RBG[h]
        rowsel = np.concatenate(
            [np.arange(g * 128, (g + 1) * 128) for g in rbgs]
        )
        ones_i = np.ones(RB * 128, bf)
        ones_j = np.ones(NB, bf)
        rowsL, rowsR = [], []
        for ax in range(3):
            rowsL += [hi[rowsel, ax], hi[rowsel, ax], lo[rowsel, ax]]
            rowsR += [
                (-2.0 * hi[:, ax].astype(np.float32)).astype(bf),
                (-2.0 * lo[:, ax].astype(np.float32)).astype(bf),
                (-2.0 * hi[:, ax].astype(np.float32)).astype(bf),
            ]
        rowsL += [n2_hi[rowsel], n2_lo[rowsel], ones_i, ones_i]
        rowsR += [ones_j, ones_j, n2_hi, n2_lo]
        augc_np = np.ascontiguousarray(
            np.concatenate(
                [np.stack(rowsL).astype(bf), np.stack(rowsR).astype(bf)], axis=1
            )
        )

        qi = qm[rowsel]  # [RB*128, NQ] slot-ordered
        q1_np = np.ascontiguousarray(
            qi.reshape(RB, 128, NQ).transpose(1, 0, 2).reshape(128, RB * NQ)
        ).astype(np.float32)
        q2_np = np.ascontiguousarray(2.0 * q1_np)

        # diag index per slot: the diagonal sits at within-window index
        # 128*(rbg mod 4) + p (each window starts at its diagonal block).
        diag_np = np.zeros((128, RB), np.float32)
        p = np.arange(128, dtype=np.float32)
        for t in range(RB):
            g = rbgs[t]
            diag_np[:, t] = 128.0 * (g % 4) + p
        in_maps.append(
            {"augc": augc_np, "q1": q1_np, "q2": q2_np, "diag": diag_np}
        )
    return in_maps


def reduce_outputs(q, results):
    """Host-side gather: u[8,2048] per core -> pot[B].

    The kernel zeroes the diagonal exactly (select), so the self term
    sum(q^2)/(2*pi)^1.5 is added here in f64.
    """
    q = np.asarray(q, np.float32)
    pots = np.zeros(B, np.float64)
    for core in range(NCORES):
        b = core // 2
        u = results[core]["uout"].astype(np.float64)
        qm = q[b * NB : (b + 1) * NB].astype(np.float64)
        pots[b] += (u * qm.T).sum()
    pots = pots / (4.0 * np.pi)
    for b in range(B):
        qm = q[b * NB : (b + 1) * NB].astype(np.float64)
        pots[b] += (qm**2).sum() / ((2.0 * np.pi) ** 1.5)
    return (pots * NORM_FACTOR).astype(np.float32)


def kernel(q, r, batch):
    global _compiled
    if _compiled is None:
        try:
            _compiled = build_nc(psa_bufs=4)
        except Exception:
            _compiled = build_nc(psa_bufs=2)
    from concourse import bass_utils

    in_maps = make_in_maps(q, r)
    last_err = None
    for attempt in range(3):
        try:
            res = bass_utils.run_bass_kernel_spmd(
                _compiled, in_maps, core_ids=list(range(NCORES))
            )
            return reduce_outputs(q, res.results)
        except Exception as e:  # transient device errors: back off and retry
            last_err = e
            import time

            time.sleep(15 * (attempt + 1))
    raise last_err
